# revision 15
# baseline (speedup 1.0000x reference)
"""Trainium2 Bass kernel for the fused cross-head attention block.

Problem shapes (hardcoded):
  x_c, x_t: [8, 256, 128, 128] f32; Wq/Wk/Wv/Wo: [256, 256]; biases [256].
  out: [8, 256, 128, 128] f32.

Math per sample (C=256, nh=8, hd=32, N=H*W=16384 tokens):
  x = x_c + x_t                                    (channel-major [C, N])
  q/k/v = per-token linear projections
  per token: dot[h,g] = q_h . k_g * hd^-0.5  (8x8 gram across heads)
             attn = softmax_g(dot);  o_h = sum_g attn[h,g] v_g
  out = Wo @ o (+ bo)                              (channel-major [C, N])

Sharding: pure data parallel, one sample per NeuronCore (8 cores).

Kernel design (per core, per 128-token tile):
  - PE pass 1 with x-block as the *stationary* operand and a precomputed
    [256, 768] concat(WqT', WkT', WvT) as the moving operand: q,k,v come out
    token-major in PSUM. q,k use head-major permuted output channels
    (h*32+c); v keeps the natural (c,g) interleaved order so the numerator
    multiply has unit-stride innermost g.
  - one ACT copy PSUM->SBUF downcasts qkv to bf16.
  - gram: DVE bf16 multiply with broadcast APs [p, h, g, c] + bf16 add-tree
    over c; exp on ACT (scale folded); denominator reduce + reciprocal and
    attn-normalization on DVE; numerator: bf16 multiply [p, h, c, g] +
    add-tree over g -> o' token-major (head-major channels).
  - PE transpose of o' (bf16) + out-projection with WoT stationary ->
    fp32 PSUM, DMA'd straight to DRAM channel-major.
"""

import os
import sys

if "/opt/trn_rl_repo" not in sys.path:
    sys.path.insert(0, "/opt/trn_rl_repo")

from contextlib import ExitStack

import ml_dtypes
import numpy as np

import concourse.bass as bass
import concourse.bacc as bacc
import concourse.tile as tile
from concourse import mybir
from concourse.bass_utils import run_bass_kernel_spmd
from concourse.masks import make_identity

B, C, HH, WW = 8, 256, 128, 128
NH, HD = 8, 32
N = HH * WW  # 16384 tokens per sample
TT = 128  # tokens per tile
SCALE = float(HD) ** -0.5

F32 = mybir.dt.float32
F32R = mybir.dt.float32r
BF16 = mybir.dt.bfloat16
F16 = mybir.dt.float16


try:
    from kernel_v3 import build_kernel  # dev: v2 implementation
except ImportError:
    build_kernel = None

_PERM = np.array([c * NH + h for h in range(NH) for c in range(HD)])


def _prep_weights(Wq, bq, Wk, bk, Wv, bv, Wo, bo):
    wq_p = Wq[_PERM]
    wk_p = Wk[_PERM]
    wqkv = np.concatenate([wq_p.T, wk_p.T, Wv.T], axis=1)
    wqkv = np.ascontiguousarray(wqkv).astype(np.float16)
    wot = np.ascontiguousarray(Wo.T).astype(np.float16)
    bqkv = np.concatenate([bq[_PERM], bk[_PERM], bv]).astype(np.float16)[None]
    bo_a = bo.astype(np.float32)[None]
    return wqkv, wot, bqkv, bo_a


def kernel(x_c, x_t, Wq, bq, Wk, bk, Wv, bv, Wo, bo):
    x_c = np.asarray(x_c, dtype=np.float32)
    x_t = np.asarray(x_t, dtype=np.float32)
    wqkv, wot, bqkv, bo_a = _prep_weights(
        np.asarray(Wq, np.float32),
        np.asarray(bq, np.float32),
        np.asarray(Wk, np.float32),
        np.asarray(bk, np.float32),
        np.asarray(Wv, np.float32),
        np.asarray(bv, np.float32),
        np.asarray(Wo, np.float32),
        np.asarray(bo, np.float32),
    )
    has_qkv_bias = bool(np.any(bqkv))
    has_o_bias = bool(np.any(bo_a))
    nc = build_kernel(has_qkv_bias=has_qkv_bias, has_o_bias=has_o_bias)

    in_maps = []
    for b in range(B):
        in_maps.append(
            {
                "xc": np.ascontiguousarray(x_c[b].reshape(C, N)),
                "xt": np.ascontiguousarray(x_t[b].reshape(C, N)),
                "wqkv": wqkv,
                "wot": wot,
                "bqkv": bqkv,
                "bo": bo_a,
            }
        )
    res = run_bass_kernel_spmd(nc, in_maps, list(range(B)))
    outs = []
    for b in range(B):
        outs.append(np.asarray(res.results[b]["out"]).reshape(C, HH, WW))
    return np.stack(outs).astype(np.float32)


def _install_ntff_shim():
    """Recreate the missing antenv.axon_hooks module + ctypes NTFF hook
    (mirrors trn_agent_boot.trn_boot's degraded-silently path)."""
    import contextlib
    import ctypes
    import types

    try:
        from antenv.axon_hooks import get_axon_ntff_profile_hook  # noqa: F401

        return True
    except ImportError:
        pass
    import antenv

    mod = types.ModuleType("antenv.axon_hooks")
    mod._hook = None

    def set_axon_ntff_profile_hook(h):
        mod._hook = h

    def get_axon_ntff_profile_hook():
        return mod._hook

    mod.set_axon_ntff_profile_hook = set_axon_ntff_profile_hook
    mod.get_axon_ntff_profile_hook = get_axon_ntff_profile_hook
    sys.modules["antenv.axon_hooks"] = mod
    antenv.axon_hooks = mod

    so_path = "/opt/axon/libaxon_pjrt.so"
    if not os.path.exists(so_path):
        return False
    lib = ctypes.CDLL(so_path)
    if not hasattr(lib, "axon_start_nrt_profile"):
        return False
    lib.axon_start_nrt_profile.argtypes = [
        ctypes.POINTER(ctypes.c_int64),
        ctypes.c_size_t,
    ]
    lib.axon_start_nrt_profile.restype = ctypes.c_int64
    lib.axon_stop_nrt_profile.argtypes = [ctypes.c_char_p]
    lib.axon_stop_nrt_profile.restype = ctypes.c_int64

    @contextlib.contextmanager
    def _hook(output_dir, device_ids):
        import jax

        jax.devices()
        if device_ids:
            ids = (ctypes.c_int64 * len(device_ids))(*device_ids)
            rc = lib.axon_start_nrt_profile(ids, len(device_ids))
        else:
            rc = lib.axon_start_nrt_profile(None, 0)
        if rc != 0:
            raise RuntimeError(f"axon_start_nrt_profile rc={rc}")
        try:
            yield
        finally:
            n = lib.axon_stop_nrt_profile(str(output_dir).encode())
            print(f"profile: {n} file(s) written to {output_dir}")

    set_axon_ntff_profile_hook(_hook)
    return True


def profile_run(inputs_np):
    """Run once more with NTFF tracing on core 0; return exec_time_ns."""
    import concourse.bass_utils as bu

    _install_ntff_shim()
    bu.upload_artifacts = lambda d: "local://" + d  # no S3 in this container
    x_c = np.asarray(inputs_np["x_c"], np.float32)
    x_t = np.asarray(inputs_np["x_t"], np.float32)
    wqkv, wot, bqkv, bo_a = _prep_weights(
        *[
            np.asarray(inputs_np[k], np.float32)
            for k in ("Wq", "bq", "Wk", "bk", "Wv", "bv", "Wo", "bo")
        ]
    )
    nc = build_kernel(
        has_qkv_bias=bool(np.any(bqkv)), has_o_bias=bool(np.any(bo_a))
    )
    in_maps = []
    for b in range(B):
        in_maps.append(
            {
                "xc": np.ascontiguousarray(x_c[b].reshape(C, N)),
                "xt": np.ascontiguousarray(x_t[b].reshape(C, N)),
                "wqkv": wqkv,
                "wot": wot,
                "bqkv": bqkv,
                "bo": bo_a,
            }
        )
    res = run_bass_kernel_spmd(nc, in_maps, list(range(B)), trace=True)
    return res.exec_time_ns


if __name__ == "__main__":
    rng = np.random.default_rng(0)
    ins = {
        "x_c": rng.standard_normal((B, C, HH, WW), dtype=np.float32),
        "x_t": rng.standard_normal((B, C, HH, WW), dtype=np.float32),
        "Wq": (rng.standard_normal((C, C)) * 0.02).astype(np.float32),
        "bq": np.zeros(C, np.float32),
        "Wk": (rng.standard_normal((C, C)) * 0.02).astype(np.float32),
        "bk": np.zeros(C, np.float32),
        "Wv": (rng.standard_normal((C, C)) * 0.02).astype(np.float32),
        "bv": np.zeros(C, np.float32),
        "Wo": (rng.standard_normal((C, C)) * 0.02).astype(np.float32),
        "bo": np.zeros(C, np.float32),
    }
    out = kernel(**ins)
    print(out.shape, out.dtype)


# revision 16
# speedup vs baseline: 1.3067x; 1.3067x over previous
"""Trainium2 Bass kernel for the fused cross-head attention block.

Problem shapes (hardcoded):
  x_c, x_t: [8, 256, 128, 128] f32; Wq/Wk/Wv/Wo: [256, 256]; biases [256].
  out: [8, 256, 128, 128] f32.

Math per sample (C=256, nh=8, hd=32, N=H*W=16384 tokens):
  x = x_c + x_t                                    (channel-major [C, N])
  q/k/v = per-token linear projections
  per token: dot[h,g] = q_h . k_g * hd^-0.5  (8x8 gram across heads)
             attn = softmax_g(dot);  o_h = sum_g attn[h,g] v_g
  out = Wo @ o (+ bo)                              (channel-major [C, N])

Sharding: pure data parallel, one sample per NeuronCore (8 cores).

Kernel design (per core, per 128-token tile):
  - PE pass 1 with x-block as the *stationary* operand and a precomputed
    [256, 768] concat(WqT', WkT', WvT) as the moving operand: q,k,v come out
    token-major in PSUM. q,k use head-major permuted output channels
    (h*32+c); v keeps the natural (c,g) interleaved order so the numerator
    multiply has unit-stride innermost g.
  - one ACT copy PSUM->SBUF downcasts qkv to bf16.
  - gram: DVE bf16 multiply with broadcast APs [p, h, g, c] + bf16 add-tree
    over c; exp on ACT (scale folded); denominator reduce + reciprocal and
    attn-normalization on DVE; numerator: bf16 multiply [p, h, c, g] +
    add-tree over g -> o' token-major (head-major channels).
  - PE transpose of o' (bf16) + out-projection with WoT stationary ->
    fp32 PSUM, DMA'd straight to DRAM channel-major.
"""

import os
import sys

if "/opt/trn_rl_repo" not in sys.path:
    sys.path.insert(0, "/opt/trn_rl_repo")

from contextlib import ExitStack

import ml_dtypes
import numpy as np

import concourse.bass as bass
import concourse.bacc as bacc
import concourse.tile as tile
from concourse import mybir
from concourse.bass_utils import run_bass_kernel_spmd
from concourse.masks import make_identity

B, C, HH, WW = 8, 256, 128, 128
NH, HD = 8, 32
N = HH * WW  # 16384 tokens per sample
TT = 128  # tokens per tile
SCALE = float(HD) ** -0.5

F32 = mybir.dt.float32
F32R = mybir.dt.float32r
BF16 = mybir.dt.bfloat16
F16 = mybir.dt.float16


try:
    from kernel_v4 import build_kernel  # dev: v2 implementation
except ImportError:
    build_kernel = None

_PERM = np.array([c * NH + h for h in range(NH) for c in range(HD)])


def _prep_weights(Wq, bq, Wk, bk, Wv, bv, Wo, bo):
    wq_p = Wq[_PERM]
    wk_p = Wk[_PERM]
    wqkv = np.concatenate([wq_p.T, wk_p.T, Wv.T], axis=1)
    wqkv = np.ascontiguousarray(wqkv).astype(np.float16)
    wot = np.ascontiguousarray(Wo.T).astype(np.float16)
    bqkv = np.concatenate([bq[_PERM], bk[_PERM], bv]).astype(np.float16)[None]
    bo_a = bo.astype(np.float32)[None]
    return wqkv, wot, bqkv, bo_a


def kernel(x_c, x_t, Wq, bq, Wk, bk, Wv, bv, Wo, bo):
    x_c = np.asarray(x_c, dtype=np.float32)
    x_t = np.asarray(x_t, dtype=np.float32)
    wqkv, wot, bqkv, bo_a = _prep_weights(
        np.asarray(Wq, np.float32),
        np.asarray(bq, np.float32),
        np.asarray(Wk, np.float32),
        np.asarray(bk, np.float32),
        np.asarray(Wv, np.float32),
        np.asarray(bv, np.float32),
        np.asarray(Wo, np.float32),
        np.asarray(bo, np.float32),
    )
    has_qkv_bias = bool(np.any(bqkv))
    has_o_bias = bool(np.any(bo_a))
    nc = build_kernel(has_qkv_bias=has_qkv_bias, has_o_bias=has_o_bias)

    in_maps = []
    for b in range(B):
        in_maps.append(
            {
                "xc": np.ascontiguousarray(x_c[b].reshape(C, N)),
                "xt": np.ascontiguousarray(x_t[b].reshape(C, N)),
                "wqkv": wqkv,
                "wot": wot,
                "bqkv": bqkv,
                "bo": bo_a,
            }
        )
    res = run_bass_kernel_spmd(nc, in_maps, list(range(B)))
    outs = []
    for b in range(B):
        outs.append(np.asarray(res.results[b]["out"]).reshape(C, HH, WW))
    return np.stack(outs).astype(np.float32)


def _install_ntff_shim():
    """Recreate the missing antenv.axon_hooks module + ctypes NTFF hook
    (mirrors trn_agent_boot.trn_boot's degraded-silently path)."""
    import contextlib
    import ctypes
    import types

    try:
        from antenv.axon_hooks import get_axon_ntff_profile_hook  # noqa: F401

        return True
    except ImportError:
        pass
    import antenv

    mod = types.ModuleType("antenv.axon_hooks")
    mod._hook = None

    def set_axon_ntff_profile_hook(h):
        mod._hook = h

    def get_axon_ntff_profile_hook():
        return mod._hook

    mod.set_axon_ntff_profile_hook = set_axon_ntff_profile_hook
    mod.get_axon_ntff_profile_hook = get_axon_ntff_profile_hook
    sys.modules["antenv.axon_hooks"] = mod
    antenv.axon_hooks = mod

    so_path = "/opt/axon/libaxon_pjrt.so"
    if not os.path.exists(so_path):
        return False
    lib = ctypes.CDLL(so_path)
    if not hasattr(lib, "axon_start_nrt_profile"):
        return False
    lib.axon_start_nrt_profile.argtypes = [
        ctypes.POINTER(ctypes.c_int64),
        ctypes.c_size_t,
    ]
    lib.axon_start_nrt_profile.restype = ctypes.c_int64
    lib.axon_stop_nrt_profile.argtypes = [ctypes.c_char_p]
    lib.axon_stop_nrt_profile.restype = ctypes.c_int64

    @contextlib.contextmanager
    def _hook(output_dir, device_ids):
        import jax

        jax.devices()
        if device_ids:
            ids = (ctypes.c_int64 * len(device_ids))(*device_ids)
            rc = lib.axon_start_nrt_profile(ids, len(device_ids))
        else:
            rc = lib.axon_start_nrt_profile(None, 0)
        if rc != 0:
            raise RuntimeError(f"axon_start_nrt_profile rc={rc}")
        try:
            yield
        finally:
            n = lib.axon_stop_nrt_profile(str(output_dir).encode())
            print(f"profile: {n} file(s) written to {output_dir}")

    set_axon_ntff_profile_hook(_hook)
    return True


def profile_run(inputs_np):
    """Run once more with NTFF tracing on core 0; return exec_time_ns."""
    import concourse.bass_utils as bu

    _install_ntff_shim()
    bu.upload_artifacts = lambda d: "local://" + d  # no S3 in this container
    x_c = np.asarray(inputs_np["x_c"], np.float32)
    x_t = np.asarray(inputs_np["x_t"], np.float32)
    wqkv, wot, bqkv, bo_a = _prep_weights(
        *[
            np.asarray(inputs_np[k], np.float32)
            for k in ("Wq", "bq", "Wk", "bk", "Wv", "bv", "Wo", "bo")
        ]
    )
    nc = build_kernel(
        has_qkv_bias=bool(np.any(bqkv)), has_o_bias=bool(np.any(bo_a))
    )
    in_maps = []
    for b in range(B):
        in_maps.append(
            {
                "xc": np.ascontiguousarray(x_c[b].reshape(C, N)),
                "xt": np.ascontiguousarray(x_t[b].reshape(C, N)),
                "wqkv": wqkv,
                "wot": wot,
                "bqkv": bqkv,
                "bo": bo_a,
            }
        )
    res = run_bass_kernel_spmd(nc, in_maps, list(range(B)), trace=True)
    return res.exec_time_ns


if __name__ == "__main__":
    rng = np.random.default_rng(0)
    ins = {
        "x_c": rng.standard_normal((B, C, HH, WW), dtype=np.float32),
        "x_t": rng.standard_normal((B, C, HH, WW), dtype=np.float32),
        "Wq": (rng.standard_normal((C, C)) * 0.02).astype(np.float32),
        "bq": np.zeros(C, np.float32),
        "Wk": (rng.standard_normal((C, C)) * 0.02).astype(np.float32),
        "bk": np.zeros(C, np.float32),
        "Wv": (rng.standard_normal((C, C)) * 0.02).astype(np.float32),
        "bv": np.zeros(C, np.float32),
        "Wo": (rng.standard_normal((C, C)) * 0.02).astype(np.float32),
        "bo": np.zeros(C, np.float32),
    }
    out = kernel(**ins)
    print(out.shape, out.dtype)
